# revision 1
# baseline (speedup 1.0000x reference)
"""Trainium2 Bass kernel for the FFT-stacked hyperbolic-BN MLP block.

Math notes (why the device kernel is so simple):

  reference: h  = relu(BN(x@W1 + b1))
             u  = logmap_c(h)          (Poincare ball, c=0.001)
             v  = Re(ifft(fft(u) * H_eff)),  H_eff = exp(L*log(g_real + i g_imag))
             y  = expmap_c(v)
             h3 = relu(BN(alpha*y + beta_p*h))
             out= h3@W2 + b2

  * b1 cancels inside batchnorm (mean subtraction), so it is dropped.
  * With H_eff == 1 (the case whenever g_real==1, g_imag==0, since
    exp(L*log(1)) == 1 exactly in complex fp32), the fft chain is the
    identity:  v == u.  Then expmap(logmap(h)) collapses:
       scn = clip(sc*|h|, EPS, 1-1e-5)
       u = artanh(scn) * h / max(sc*|h|, EPS);  y = tanh(sc*|u|) u / (sc*|u|)
    - unclipped rows: tanh(artanh(z)) == z  =>  y == h
    - clipped rows (sc*|h| > 1-1e-5):  y == (1-1e-5) * h / (sc*|h|)
    so y = h * min(1, (1-1e-5)/(sc*|h|)) exactly, and
       alpha*y + beta_p*h = (alpha*min(1,R/|h|) + beta_p) * h  =: g(row) * h.
  * More generally the fft chain is a circulant convolution with the real
    kernel Re(ifft(H_eff)); we check at run time that this kernel is a delta
    (it is, for the shipped inputs) and otherwise fall back to a faithful
    numpy implementation of the whole reference.

Device pipeline per core (batch-sharded, 1024 rows/core, 8 cores):
  P1 : Z = xT.T @ W1 tile-wise (PE, f32r), per-column sum/sumsq fused into the
       PSUM->SBUF evacuation on ACT (accum_out); Z spilled to DRAM.
  CC1: AllReduce 64KB of BN1 stats across the 8 cores; scale/bias from
       mu/var on-chip.
  P2 : reload Z, h = relu(scale*z+bias) in one ACT op; h kept resident in
       SBUF; row-norms^2 accumulated with a ones-vector matmul on PE
       (partition-dim reduction).
  P2b: g = alpha*min(R/|h|,1)+beta_p on one partition; broadcast via DMA.
  P2c: h2 = g*h (DVE); BN2 stats (DVE reduce + ACT square-accum).
  CC2: AllReduce BN2 stats; scale2/bias2.
  P2d: h3 = relu(scale2*h2+bias2) written f32r in place.
  P3 : out = h3 @ W2 + b2 (PE f32r, h3 slices as stationary), bias added
       during PSUM evacuation (DVE), streamed out.
"""

import os
import sys

sys.path.insert(0, "/opt/trn_rl_repo")

import numpy as np

B_FULL = 8192
D_IN = 3072
D_H = 4096
D_OUT = 1000
N_CORES = 8
B_SH = B_FULL // N_CORES          # 1024 rows per core
KT = D_IN // 128                  # 24 k-tiles
HT = D_H // 128                   # 32 h-tiles
BT = B_SH // 128                  # 8 row-tiles per core

C_CURV = 0.001
EPS = 1e-7
BN_EPS = 1e-5
L_EXP = 100000000
SC = float(np.sqrt(np.float32(C_CURV)))
R_CLIP = float((1.0 - 1e-5) / SC)   # radius above which rows get rescaled

MM_MODE = os.environ.get("BASS_MM_MODE", "f32r")   # "f32r" | "f32"

_BUILD_CACHE = {}


def _filter_kernel(g_real, g_imag):
    """Real circulant kernel of the fft->*H_eff->ifft chain (complex64 math,
    mirroring the reference)."""
    H = g_real.astype(np.complex64) + 1j * g_imag.astype(np.complex64)
    H_eff = np.exp(np.complex64(L_EXP) * np.log(H))
    return np.fft.ifft(H_eff)


def _np_reference(x, W1, b1, gamma1, beta1, g_real, g_imag, alpha, beta_p,
                  gamma2, beta2, W2, b2):
    """Faithful numpy fallback for non-delta spectral filters."""
    def bn(a, gamma, beta):
        mu = a.mean(0)
        var = a.var(0)
        return gamma * (a - mu) / np.sqrt(var + BN_EPS) + beta

    def logmap(h):
        n = np.linalg.norm(h, axis=1, keepdims=True)
        scn = np.clip(SC * n, EPS, 1.0 - 1e-5)
        return np.arctanh(scn) * h / np.maximum(SC * n, EPS)

    def expmap(v):
        n = np.maximum(np.linalg.norm(v, axis=1, keepdims=True), EPS)
        return np.tanh(SC * n) * v / (SC * n)

    h = np.maximum(bn(x @ W1 + b1, gamma1, beta1), 0.0)
    u = logmap(h)
    U = np.fft.fft(u, axis=1)
    H = g_real.astype(np.complex64) + 1j * g_imag.astype(np.complex64)
    H_eff = np.exp(np.complex64(L_EXP) * np.log(H))
    v = np.real(np.fft.ifft(U * H_eff[None, :], axis=1)).astype(np.float32)
    y = expmap(v)
    h2 = alpha * y + beta_p * h
    h3 = np.maximum(bn(h2, gamma2, beta2), 0.0)
    return (h3 @ W2 + b2).astype(np.float32)


def _build(mm_mode):
    import concourse.bacc as bacc
    import concourse.mybir as mybir
    import concourse.tile as tile

    skip_cc = os.environ.get("BASS_SKIP_CC", "0") == "1"
    ht_lim = int(os.environ.get("BASS_HT_LIM", str(HT)))
    phase_lim = int(os.environ.get("BASS_PHASE_LIM", "4"))
    p2_lim = os.environ.get("BASS_P2_LIM", "d")

    f32 = mybir.dt.float32
    f32r = mybir.dt.float32r
    mmdt = f32r if mm_mode == "f32r" else f32
    AFT = mybir.ActivationFunctionType
    ALU = mybir.AluOpType

    nc = bacc.Bacc("TRN2", target_bir_lowering=False, debug=False,
                   num_devices=N_CORES)

    xT = nc.dram_tensor("xT", [D_IN, B_SH], f32, kind="ExternalInput")
    # host passes W1 repacked: W1p[p, hg, kt, j] = W1[kt*128+p, hg*256+j]
    W1 = nc.dram_tensor("W1", [128, HT // 2, KT, 256], f32,
                        kind="ExternalInput")
    # host passes these pre-transposed to [128, HT] (partition-major)
    gamma1 = nc.dram_tensor("gamma1", [128, HT], f32, kind="ExternalInput")
    beta1 = nc.dram_tensor("beta1", [128, HT], f32, kind="ExternalInput")
    gamma2 = nc.dram_tensor("gamma2", [128, HT], f32, kind="ExternalInput")
    beta2 = nc.dram_tensor("beta2", [128, HT], f32, kind="ExternalInput")
    alpha_e = nc.dram_tensor("alpha", [1], f32, kind="ExternalInput")
    beta_p_e = nc.dram_tensor("beta_p", [1], f32, kind="ExternalInput")
    W2 = nc.dram_tensor("W2", [D_H, D_OUT], f32, kind="ExternalInput")
    b2 = nc.dram_tensor("b2", [D_OUT], f32, kind="ExternalInput")
    out = nc.dram_tensor("out", [B_SH, D_OUT], f32, kind="ExternalOutput")

    z_dram = nc.dram_tensor("z_scr", [HT, 128, B_SH], f32)
    cc1_ins = [nc.dram_tensor(f"cc1_in{i}", [128, 32], f32) for i in range(4)]
    cc1_outs = [nc.dram_tensor(f"cc1_out{i}", [128, 32], f32,
                               addr_space="Shared") for i in range(4)]
    cc2_ins = [nc.dram_tensor(f"cc2_in{i}", [128, 32], f32) for i in range(2)]
    cc2_outs = [nc.dram_tensor(f"cc2_out{i}", [128, 32], f32,
                               addr_space="Shared") for i in range(2)]
    g_dram = nc.dram_tensor("g_scr", [B_SH], f32)

    bitcast_loads = os.environ.get("BASS_BITCAST_LOADS", "1") == "1"
    if mm_mode != "f32r":
        cast_dma, castf = nc.sync, (lambda ap: ap)
    elif bitcast_loads:
        cast_dma, castf = nc.sync, (lambda ap: ap.bitcast(f32r))
    else:
        cast_dma, castf = nc.gpsimd, (lambda ap: ap)

    with tile.TileContext(nc) as tc:
        with tc.tile_pool(name="consts", bufs=1) as consts:
            g1 = consts.tile([128, HT], f32)
            bt1 = consts.tile([128, HT], f32)
            g2 = consts.tile([128, HT], f32)
            bt2 = consts.tile([128, HT], f32)
            nc.sync.dma_start(out=g1[:], in_=gamma1[:])
            nc.sync.dma_start(out=bt1[:], in_=beta1[:])
            nc.sync.dma_start(out=g2[:], in_=gamma2[:])
            nc.sync.dma_start(out=bt2[:], in_=beta2[:])
            b2b = consts.tile([128, D_OUT], f32)
            nc.sync.dma_start(out=b2b[:], in_=b2[None, :].to_broadcast([128, D_OUT]))
            ab_sb = consts.tile([1, 2], f32)
            nc.sync.dma_start(out=ab_sb[0:1, 0:1], in_=alpha_e[None, :])
            nc.sync.dma_start(out=ab_sb[0:1, 1:2], in_=beta_p_e[None, :])
            ones_f32 = consts.tile([128, 1], f32)
            nc.vector.memset(ones_f32[:], 1.0)
            ones_col = consts.tile([128, 1], mmdt)
            nc.scalar.activation(ones_col[:], ones_f32[:], AFT.Identity)
            eps_col = consts.tile([128, 1], f32)
            nc.vector.memset(eps_col[:], BN_EPS)

            # per-quarter stats: [0:8]=sum(bc0) [8:16]=sum(bc1)
            #                    [16:24]=sq(bc0) [24:32]=sq(bc1)
            stats1h = [consts.tile([128, 32], f32, name=f"stats1h{i}")
                       for i in range(4)]
            # per-half BN2 stats: [0:16]=sum, [16:32]=sq
            stats2h = [consts.tile([128, 32], f32, name=f"stats2h{i}")
                       for i in range(2)]
            scale1h = [consts.tile([128, 8], f32, name=f"scale1h{i}")
                       for i in range(4)]
            bias1h = [consts.tile([128, 8], f32, name=f"bias1h{i}")
                      for i in range(4)]
            scale2h = [consts.tile([128, 16], f32, name=f"scale2h{i}")
                       for i in range(2)]
            bias2h = [consts.tile([128, 16], f32, name=f"bias2h{i}")
                      for i in range(2)]
            tmps2 = [consts.tile([128, 16], f32, name=f"tmps2_{i}")
                     for i in range(3)]
            tmps = [consts.tile([128, 16], f32, name=f"tmps{i}")
                    for i in range(6)]
            gvec = consts.tile([1, B_SH], f32)
            gb = consts.tile([128, B_SH], f32)

            def bn_coeffs(s_lo, s_hi, scl, bia, t1, t2, t3, gbase, bbase,
                          off, w=16):
                # mu = s_lo/B ; var = s_hi/B - mu^2
                nc.vector.tensor_scalar_mul(t1[:], s_lo, 1.0 / B_FULL)   # mu
                nc.vector.tensor_scalar_mul(t2[:], s_hi, 1.0 / B_FULL)   # E[z^2]
                nc.vector.tensor_mul(t3[:], t1[:], t1[:])                # mu^2
                nc.vector.tensor_sub(t2[:], t2[:], t3[:])                # var
                nc.scalar.activation(t2[:], t2[:], AFT.Sqrt, bias=eps_col[:])
                nc.vector.reciprocal(t2[:], t2[:])                       # rstd
                nc.vector.tensor_mul(scl[:], gbase[:, off:off + w], t2[:])
                nc.vector.tensor_mul(t3[:], t1[:], scl[:])
                nc.vector.tensor_sub(bia[:], bbase[:, off:off + w], t3[:])

            # zin opened before P1 so P2's first z reloads prefetch during
            # P1's tail (zin closes after the P2/P3 block below)
            zin_cm = tc.tile_pool(name="zin", bufs=2)
            zip_ = zin_cm.__enter__()

            # ---------------- P1: Z = x @ W1, stats fused ----------------
            with tc.tile_pool(name="xt", bufs=1) as xtp, \
                 tc.tile_pool(name="w1", bufs=2) as w1p, \
                 tc.tile_pool(name="zst", bufs=4) as zp, \
                 tc.tile_pool(name="ps1", bufs=6, space="PSUM") as pp1:
                xts = [xtp.tile([128, B_SH], mmdt, name=f"xt{kt}")
                       for kt in range(KT)]

                def load_w1(htg):
                    # one 24KB contiguous run per partition (host-repacked W1)
                    w1t = w1p.tile([128, KT, 256], mmdt, name="w1t")
                    cast_dma.dma_start(out=w1t[:], in_=castf(W1[:, htg]))
                    return w1t

                # first W1 slice before the big xT load so mm1 can start as
                # soon as the first k-chunks of xT land
                w1_next = load_w1(0)
                dma_engs = [nc.sync, nc.scalar]
                for kt0 in range(KT):
                    dma_engs[kt0 % len(dma_engs)].dma_start(
                        out=xts[kt0][:],
                        in_=castf(xT.rearrange("(kt kp) b -> kp kt b", kp=128)[
                            :, kt0, :]))
                for ht in range(ht_lim):
                    if ht % 2 == 0:
                        w1g = w1_next
                        if ht + 2 < ht_lim:
                            w1_next = load_w1((ht + 2) // 2)
                    w1off = (ht % 2) * 128
                    for bc in range(2):
                        ps = pp1.tile([128, 512], f32, tag="ps")
                        for kt in range(KT):
                            nc.tensor.matmul(
                                ps[:], w1g[:, kt, w1off:w1off + 128],
                                xts[kt][:, bc * 512:(bc + 1) * 512],
                                start=(kt == 0), stop=(kt == KT - 1))
                        zt = zp.tile([128, 512], f32, tag="zt")
                        sq = zp.tile([128, 512], f32, tag="sq")
                        half, hh = divmod(ht, 8)
                        st = stats1h[half]
                        nc.scalar.activation(
                            zt[:], ps[:], AFT.Copy,
                            accum_out=st[:, bc * 8 + hh:bc * 8 + hh + 1])
                        nc.scalar.activation(
                            sq[:], ps[:], AFT.Square,
                            accum_out=st[:, 16 + bc * 8 + hh:16 + bc * 8 + hh + 1])
                        nc.sync.dma_start(
                            out=z_dram[ht, :, bc * 512:(bc + 1) * 512], in_=zt[:])
                    if ht % 8 == 7 and phase_lim >= 2:
                        half = ht // 8
                        t0_, t1_, t2_ = (tmps[3 * (half % 2)],
                                         tmps[3 * (half % 2) + 1],
                                         tmps[3 * (half % 2) + 2])
                        nc.sync.dma_start(out=cc1_ins[half][:],
                                          in_=stats1h[half][:])
                        if skip_cc:
                            nc.sync.dma_start(out=cc1_outs[half][:],
                                              in_=cc1_ins[half][:])
                            nc.vector.tensor_scalar_mul(
                                stats1h[half][:], stats1h[half][:],
                                float(N_CORES))
                        else:
                            nc.gpsimd.collective_compute(
                                "AllReduce", mybir.AluOpType.add,
                                replica_groups=[list(range(N_CORES))],
                                ins=[cc1_ins[half][:]],
                                outs=[cc1_outs[half][:]])
                            nc.sync.dma_start(out=stats1h[half][:],
                                              in_=cc1_outs[half][:])
                        nc.vector.tensor_add(
                            t0_[:, 0:8], stats1h[half][:, 0:8],
                            stats1h[half][:, 8:16])
                        nc.vector.tensor_add(
                            t1_[:, 0:8], stats1h[half][:, 16:24],
                            stats1h[half][:, 24:32])
                        bn_coeffs(t0_[:, 0:8], t1_[:, 0:8],
                                  scale1h[half], bias1h[half],
                                  t2_[:, 0:8], t0_[:, 0:8],
                                  t1_[:, 0:8], g1, bt1, half * 8, w=8)

            if phase_lim == 1:
                with tc.tile_pool(name="dummy", bufs=1) as dmy:
                    dt_ = dmy.tile([128, 128], f32)
                    nc.sync.dma_start(out=dt_[:], in_=z_dram[0, :, 0:128])
                    nc.sync.dma_start(out=out[0:128, 0:128], in_=dt_[:])

            if phase_lim >= 2:
                # ------------ P2: h = relu(bn1(z)), norms, g, bn2 --------
                if p2_lim != "a":
                  with tc.tile_pool(name="h", bufs=1) as hp, \
                     tc.tile_pool(name="sq2", bufs=4) as sqp:
                    h_sb = hp.tile([128, HT, B_SH], mmdt)
                    with tc.tile_pool(name="psn", bufs=1, space="PSUM") as ppn:
                        n2ps = [ppn.tile([1, 512], f32, tag=f"n2_{i}",
                                         name=f"n2_{i}") for i in range(2)]
                        zengs = [nc.sync, nc.scalar]
                        for hp2 in range(0, ht_lim, 2):
                            # two h-tiles per reload: halves the per-DMA
                            # fixed costs pacing this sweep
                            zt = zip_.tile([128, 2, B_SH], f32, tag="zt2")
                            zengs[(hp2 // 2) % 2].dma_start(
                                out=zt[:],
                                in_=z_dram[hp2:hp2 + 2].rearrange(
                                    "t p b -> p t b"))
                            for j in range(2):
                                ht = hp2 + j
                                half, hh = divmod(ht, 8)
                                nc.scalar.activation(
                                    h_sb[:, ht, :], zt[:, j, :], AFT.Relu,
                                    bias=bias1h[half][:, hh:hh + 1],
                                    scale=scale1h[half][:, hh:hh + 1])
                                hview = h_sb[:, ht, :].bitcast(f32)
                                sq = sqp.tile([128, B_SH], mmdt, tag="sqn")
                                nc.vector.tensor_mul(sq[:], hview, hview)
                                for bc in range(2):
                                    nc.tensor.matmul(
                                        n2ps[bc][:], ones_col[:],
                                        sq[:, bc * 512:(bc + 1) * 512],
                                        start=(ht == 0),
                                        stop=(ht == ht_lim - 1))

                        # ---- P2b: g row-scales
                        nc.vector.tensor_copy(gvec[0:1, 0:512], n2ps[0][:])
                        nc.vector.tensor_copy(gvec[0:1, 512:1024], n2ps[1][:])

                    nc.scalar.activation(gvec[0:1, :], gvec[0:1, :], AFT.Sqrt)
                    nc.vector.reciprocal(gvec[0:1, :], gvec[0:1, :])
                    nc.vector.tensor_scalar(
                        out=gvec[0:1, :], in0=gvec[0:1, :],
                        scalar1=R_CLIP, scalar2=1.0, op0=ALU.mult, op1=ALU.min)
                    nc.vector.tensor_scalar(
                        out=gvec[0:1, :], in0=gvec[0:1, :],
                        scalar1=ab_sb[0:1, 0:1], scalar2=ab_sb[0:1, 1:2],
                        op0=ALU.mult, op1=ALU.add)
                    nc.sync.dma_start(out=g_dram[None, :], in_=gvec[0:1, :])
                    nc.sync.dma_start(
                        out=gb[:], in_=g_dram[None, :].to_broadcast([128, B_SH]))

                    if phase_lim >= 3:
                        # ---- P2c: h2 = g*h, BN2 stats (half chunks)
                        for ht in range(ht_lim):
                            q, hq = divmod(ht, 16)
                            h2v = h_sb[:, ht, :].bitcast(f32)
                            nc.vector.scalar_tensor_tensor(
                                out=h_sb[:, ht, :], in0=h2v, scalar=1.0,
                                in1=gb[:], op0=ALU.mult, op1=ALU.mult,
                                accum_out=stats2h[q][:, hq:hq + 1])
                            sq = sqp.tile([128, B_SH], f32, tag="sqn2")
                            nc.scalar.activation(
                                sq[:], h2v, AFT.Square,
                                accum_out=stats2h[q][:, 16 + hq:16 + hq + 1])
                            if hq == 15:
                                nc.sync.dma_start(out=cc2_ins[q][:],
                                                  in_=stats2h[q][:])
                                if skip_cc:
                                    nc.sync.dma_start(out=cc2_outs[q][:],
                                                      in_=cc2_ins[q][:])
                                    nc.vector.tensor_scalar_mul(
                                        stats2h[q][:], stats2h[q][:],
                                        float(N_CORES))
                                else:
                                    nc.gpsimd.collective_compute(
                                        "AllReduce", mybir.AluOpType.add,
                                        replica_groups=[list(range(N_CORES))],
                                        ins=[cc2_ins[q][:]],
                                        outs=[cc2_outs[q][:]])
                                    nc.sync.dma_start(
                                        out=stats2h[q][:],
                                        in_=cc2_outs[q][:])
                                bn_coeffs(stats2h[q][:, 0:16],
                                          stats2h[q][:, 16:32],
                                          scale2h[q], bias2h[q],
                                          tmps2[0], tmps2[1], tmps2[2],
                                          g2, bt2, q * 16, w=16)

                        # ---- P2d: h3 = relu(bn2(h2)) rounded in place
                        for ht in range(ht_lim):
                            q, hq = divmod(ht, 16)
                            nc.scalar.activation(
                                h_sb[:, ht, :], h_sb[:, ht, :].bitcast(f32),
                                AFT.Relu, bias=bias2h[q][:, hq:hq + 1],
                                scale=scale2h[q][:, hq:hq + 1])

                    if phase_lim == 3:
                        ot0 = zip_.tile([128, B_SH], f32, tag="zt2")
                        nc.vector.tensor_copy(ot0[:], h_sb[:, 0, :].bitcast(f32))
                        nc.sync.dma_start(out=out[0:128, 0:512], in_=ot0[:, 0:512])

                    if phase_lim >= 4:
                        # ------------ P3: out = h3 @ W2 + b2 ------------
                        with tc.tile_pool(name="w2", bufs=4) as w2p, \
                             tc.tile_pool(name="os", bufs=3) as osp, \
                             tc.tile_pool(name="ps3", bufs=1, space="PSUM") as pp3:
                            for oc, (o0, ow) in enumerate([(0, 512), (512, 488)]):
                                pss = [pp3.tile([128, 512], f32, tag=f"po{bt}",
                                                name=f"po{bt}")
                                       for bt in range(BT)]
                                for ht in range(ht_lim):
                                    w2t = w2p.tile([128, 512], mmdt, tag="w2t")
                                    cast_dma.dma_start(
                                        out=w2t[:, 0:ow],
                                        in_=castf(
                                            W2.rearrange("(t p) o -> p t o", p=128)[
                                                :, ht, o0:o0 + ow]))
                                    for bt in range(BT):
                                        nc.tensor.matmul(
                                            pss[bt][:, 0:ow],
                                            h_sb[:, ht, bt * 128:(bt + 1) * 128],
                                            w2t[:, 0:ow],
                                            start=(ht == 0),
                                            stop=(ht == ht_lim - 1))
                                for bt in range(BT):
                                    ot = osp.tile([128, 512], f32, tag="ot")
                                    nc.vector.tensor_add(
                                        ot[:, 0:ow], pss[bt][:, 0:ow],
                                        b2b[:, o0:o0 + ow])
                                    nc.sync.dma_start(
                                        out=out[bt * 128:(bt + 1) * 128,
                                                o0:o0 + ow],
                                        in_=ot[:, 0:ow])

            zin_cm.__exit__(None, None, None)

    nc.compile()
    return nc


def _get_nc(mm_mode):
    nc = _BUILD_CACHE.get(mm_mode)
    if nc is None:
        nc = _build(mm_mode)
        _BUILD_CACHE[mm_mode] = nc
    return nc


def kernel(**inputs):
    x = np.asarray(inputs["x"], np.float32)
    g_real = np.asarray(inputs["g_real"], np.float32)
    g_imag = np.asarray(inputs["g_imag"], np.float32)

    # Spectral filter must be (numerically) a delta for the fused fast path.
    ck = _filter_kernel(g_real, g_imag)
    delta = np.zeros_like(ck)
    delta[0] = 1.0
    ck_view = ck.view(np.float32) if ck.dtype == np.complex64 else ck.view(np.float64)
    if not (np.all(np.isfinite(ck_view)) and np.abs(ck - delta).max() < 1e-6):
        a = {k: np.asarray(v) for k, v in inputs.items()}
        return _np_reference(
            a["x"], a["W1"], a["b1"], a["gamma1"], a["beta1"], a["g_real"],
            a["g_imag"], float(a["alpha"][0]), float(a["beta_p"][0]),
            a["gamma2"], a["beta2"], a["W2"], a["b2"])

    from concourse.bass_utils import run_bass_kernel_spmd

    nc = _get_nc(MM_MODE)
    def _pt(v):  # [4096] -> [128, 32] partition-major
        return np.ascontiguousarray(
            np.asarray(v, np.float32).reshape(HT, 128).T)

    shared = {
        "W1": np.ascontiguousarray(
            np.asarray(inputs["W1"], np.float32)
            .reshape(KT, 128, HT // 2, 256).transpose(1, 2, 0, 3)),
        "gamma1": _pt(inputs["gamma1"]),
        "beta1": _pt(inputs["beta1"]),
        "gamma2": _pt(inputs["gamma2"]),
        "beta2": _pt(inputs["beta2"]),
        "alpha": np.ascontiguousarray(inputs["alpha"], dtype=np.float32),
        "beta_p": np.ascontiguousarray(inputs["beta_p"], dtype=np.float32),
        "W2": np.ascontiguousarray(inputs["W2"], dtype=np.float32),
        "b2": np.ascontiguousarray(inputs["b2"], dtype=np.float32),
    }
    in_maps = []
    for c in range(N_CORES):
        sh = dict(shared)
        sh["xT"] = np.ascontiguousarray(x[c * B_SH:(c + 1) * B_SH, :].T)
        in_maps.append(sh)
    res = run_bass_kernel_spmd(nc, in_maps, list(range(N_CORES)))
    return np.concatenate(
        [res.results[c]["out"] for c in range(N_CORES)], axis=0)



# revision 8
# speedup vs baseline: 1.0988x; 1.0988x over previous
"""Trainium2 Bass kernel for the FFT-stacked hyperbolic-BN MLP block.

Math notes (why the device kernel is so simple):

  reference: h  = relu(BN(x@W1 + b1))
             u  = logmap_c(h)          (Poincare ball, c=0.001)
             v  = Re(ifft(fft(u) * H_eff)),  H_eff = exp(L*log(g_real + i g_imag))
             y  = expmap_c(v)
             h3 = relu(BN(alpha*y + beta_p*h))
             out= h3@W2 + b2

  * b1 cancels inside batchnorm (mean subtraction), so it is dropped.
  * With H_eff == 1 (the case whenever g_real==1, g_imag==0, since
    exp(L*log(1)) == 1 exactly in complex fp32), the fft chain is the
    identity:  v == u, and expmap(logmap(h)) collapses to
       y = h * min(1, (1-1e-5)/(sc*|h|)),  so
       alpha*y + beta_p*h = (alpha*min(1,R/|h|) + beta_p) * h =: g(row) * h.
  * More generally the fft chain is a circulant convolution with the real
    kernel Re(ifft(H_eff)); we check at run time that this kernel is a delta
    (it is, for the shipped inputs) and otherwise fall back to a faithful
    numpy implementation of the whole reference.

Device pipeline per core (batch-sharded, 1024 rows/core, 8 cores):

  P1 (three fp8e4m3 DoubleRow passes, residual-corrected):
     z*128 = x(8)Wa + (32(x-x8))(8)(Wa/32) + (x/16)(8)(16(W*128-Wa))
     Terms are host-quantized so every PSUM accumulation carries the same
     2^7 scale; passes run small-to-large (c, b, a) so the bf16 z
     accumulator never rounds a large running value against a small term.
     Pass a's PSUM->SBUF evacuation on DVE carries accum_out (BN1 column
     sums); an ACT Square pass accumulates sum(z^2). DoubleRow processes
     2 k-tiles/instruction at 0.5 cycles/row: 4x the bf16 matmul rate.
  BN1: stats exchanged per column chunk (10|10|8|4 ht tiles) through a
     DRAM AllGather (cheaper than AllReduce in latency and off the
     critical path for all but the last chunk), summed locally on DVE,
     then h = relu(scale*z+bias) in place (bf16).
  Norms: DVE squares + PE ones-matmul partition reduction, interleaved
     into the P1 instruction stream chunk by chunk.
  g row-scales: computed on one partition, broadcast to 128 partitions
     with a rank-1 f32 matmul on PE (no DRAM round trip).
  P2: h2 = g*h on DVE (4x bf16 mode) with fused BN2 column sums;
     sum(h2^2) split across ACT/DVE. BN2 stats exchanged in 3 chunks
     (8|12|12) so P3 can start while later chunks are still in flight.
  P3: out = h3 @ W2 + b2 (bf16, h3 slices stationary), two bt passes of
     8 PSUM banks each; W2 streamed bf16; bias added on DVE during
     evacuation; pass-A rows DMA out while pass B computes.
"""

import os
import sys

sys.path.insert(0, "/opt/trn_rl_repo")

import numpy as np
import ml_dtypes

F8NP = ml_dtypes.float8_e4m3
BF16NP = ml_dtypes.bfloat16

B_FULL = 8192
D_IN = 3072
D_H = 4096
D_OUT = 1000
N_CORES = 8
B_SH = B_FULL // N_CORES          # 1024 rows per core
KT = D_IN // 128                  # 24 k-tiles
KP = KT // 2                      # 12 DoubleRow k-pairs
HT = D_H // 128                   # 32 h-tiles
HG = HT // 2                      # 16 groups of 2 ht (256 cols)
BT = B_SH // 128                  # 8 row-tiles per core

C_CURV = 0.001
EPS = 1e-7
BN_EPS = 1e-5
L_EXP = 100000000
SC = float(np.sqrt(np.float32(C_CURV)))
R_CLIP = float((1.0 - 1e-5) / SC)   # radius above which rows get rescaled

S_W = 128.0                       # power-of-2 scale on W1 (fp8 subnormal guard)
S_XB = 32.0                       # scale on the x residual term
S_WC = 16.0                       # scale on the W residual term

BN1_CHUNKS = [10, 10, 8, 4]       # ht tiles per BN1 stats exchange
BN2_CHUNKS = [8, 12, 12]          # ht tiles per BN2 stats exchange

_BUILD_CACHE = {}


def _filter_kernel(g_real, g_imag):
    """Real circulant kernel of the fft->*H_eff->ifft chain (complex64 math,
    mirroring the reference)."""
    H = g_real.astype(np.complex64) + 1j * g_imag.astype(np.complex64)
    H_eff = np.exp(np.complex64(L_EXP) * np.log(H))
    return np.fft.ifft(H_eff)


def _np_reference(x, W1, b1, gamma1, beta1, g_real, g_imag, alpha, beta_p,
                  gamma2, beta2, W2, b2):
    """Faithful numpy fallback for non-delta spectral filters."""
    def bn(a, gamma, beta):
        mu = a.mean(0)
        var = a.var(0)
        return gamma * (a - mu) / np.sqrt(var + BN_EPS) + beta

    def logmap(h):
        n = np.linalg.norm(h, axis=1, keepdims=True)
        scn = np.clip(SC * n, EPS, 1.0 - 1e-5)
        return np.arctanh(scn) * h / np.maximum(SC * n, EPS)

    def expmap(v):
        n = np.maximum(np.linalg.norm(v, axis=1, keepdims=True), EPS)
        return np.tanh(SC * n) * v / (SC * n)

    h = np.maximum(bn(x @ W1 + b1, gamma1, beta1), 0.0)
    u = logmap(h)
    U = np.fft.fft(u, axis=1)
    H = g_real.astype(np.complex64) + 1j * g_imag.astype(np.complex64)
    H_eff = np.exp(np.complex64(L_EXP) * np.log(H))
    v = np.real(np.fft.ifft(U * H_eff[None, :], axis=1)).astype(np.float32)
    y = expmap(v)
    h2 = alpha * y + beta_p * h
    h3 = np.maximum(bn(h2, gamma2, beta2), 0.0)
    return (h3 @ W2 + b2).astype(np.float32)


def _build():
    import concourse.bacc as bacc
    import concourse.mybir as mybir
    import concourse.tile as tile

    f32 = mybir.dt.float32
    bf16 = mybir.dt.bfloat16
    fp8 = mybir.dt.float8e4
    AFT = mybir.ActivationFunctionType
    ALU = mybir.AluOpType
    DR = mybir.MatmulPerfMode.DoubleRow
    GROUPS = [list(range(N_CORES))]

    nc = bacc.Bacc("TRN2", target_bir_lowering=False, debug=False,
                   num_devices=N_CORES)

    # per-core fp8 term operands, packed [128, kp, j, b]: k = kp*256+j*128+p
    x_in = [nc.dram_tensor(f"x_{t}", [128, KP, 2, B_SH], fp8,
                           kind="ExternalInput") for t in "abc"]
    # shared W1 terms, packed [128, hg, term, kp, j, c(256)]
    W1p = nc.dram_tensor("W1p", [128, HG, 3, KP, 2, 256], fp8,
                         kind="ExternalInput")
    # [4096] -> [128, 32] partition-major
    gamma1 = nc.dram_tensor("gamma1", [128, HT], f32, kind="ExternalInput")
    beta1 = nc.dram_tensor("beta1", [128, HT], f32, kind="ExternalInput")
    gamma2 = nc.dram_tensor("gamma2", [128, HT], f32, kind="ExternalInput")
    beta2 = nc.dram_tensor("beta2", [128, HT], f32, kind="ExternalInput")
    alpha_e = nc.dram_tensor("alpha", [1], f32, kind="ExternalInput")
    beta_p_e = nc.dram_tensor("beta_p", [1], f32, kind="ExternalInput")
    W2p = nc.dram_tensor("W2p", [128, HT, D_OUT], bf16, kind="ExternalInput")
    b2 = nc.dram_tensor("b2", [D_OUT], f32, kind="ExternalInput")
    out = nc.dram_tensor("out", [B_SH, D_OUT], f32, kind="ExternalOutput")

    # BN stats exchange buffers (DRAM AllGather staging)
    cc1_ins = [nc.dram_tensor(f"cc1_in{q}", [128, ch, 4], f32)
               for q, ch in enumerate(BN1_CHUNKS)]
    cc1_outs = [nc.dram_tensor(f"cc1_out{q}", [N_CORES, 128, ch, 4], f32,
                               addr_space="Shared")
                for q, ch in enumerate(BN1_CHUNKS)]
    cc2_ins = [nc.dram_tensor(f"cc2_in{q}", [128, ch, 2], f32)
               for q, ch in enumerate(BN2_CHUNKS)]
    cc2_outs = [nc.dram_tensor(f"cc2_out{q}", [N_CORES, 128, ch, 2], f32,
                               addr_space="Shared")
                for q, ch in enumerate(BN2_CHUNKS)]

    bn1_first = np.cumsum([0] + BN1_CHUNKS)[:-1]
    bn2_first = np.cumsum([0] + BN2_CHUNKS)[:-1]

    def bn1_chunk_of(ht):
        for q, f in enumerate(bn1_first):
            if f <= ht < f + BN1_CHUNKS[q]:
                return q, ht - f
        raise AssertionError

    def bn2_chunk_of(ht):
        for q, f in enumerate(bn2_first):
            if f <= ht < f + BN2_CHUNKS[q]:
                return q, ht - f
        raise AssertionError

    with tile.TileContext(nc) as tc:
        with tc.tile_pool(name="consts", bufs=1) as consts:
            g1 = consts.tile([128, HT], f32)
            bt1 = consts.tile([128, HT], f32)
            g2 = consts.tile([128, HT], f32)
            bt2 = consts.tile([128, HT], f32)
            nc.sync.dma_start(out=g1[:], in_=gamma1[:])
            nc.sync.dma_start(out=bt1[:], in_=beta1[:])
            nc.sync.dma_start(out=g2[:], in_=gamma2[:])
            nc.sync.dma_start(out=bt2[:], in_=beta2[:])
            b2b = consts.tile([128, D_OUT], f32)
            nc.sync.dma_start(out=b2b[:], in_=b2[None, :].to_broadcast([128, D_OUT]))
            ab_sb = consts.tile([1, 2], f32)
            nc.sync.dma_start(out=ab_sb[0:1, 0:1], in_=alpha_e[None, :])
            nc.sync.dma_start(out=ab_sb[0:1, 1:2], in_=beta_p_e[None, :])
            ones_f32 = consts.tile([128, 1], f32)
            nc.vector.memset(ones_f32[:], 1.0)
            ones_bf = consts.tile([128, 1], bf16)
            nc.scalar.activation(ones_bf[:], ones_f32[:], AFT.Identity)
            ones_row = consts.tile([1, 128], f32)
            nc.vector.memset(ones_row[:], 1.0)
            eps_col = consts.tile([128, 1], f32)
            nc.vector.memset(eps_col[:], BN_EPS)

            st1 = [consts.tile([128, ch, 4], f32, name=f"st1_{q}")
                   for q, ch in enumerate(BN1_CHUNKS)]
            st2 = [consts.tile([128, ch, 2], f32, name=f"st2_{q}")
                   for q, ch in enumerate(BN2_CHUNKS)]
            ag1 = [consts.tile([128, N_CORES, ch, 4], f32, name=f"ag1_{q}")
                   for q, ch in enumerate(BN1_CHUNKS)]
            ag2 = [consts.tile([128, N_CORES, ch, 2], f32, name=f"ag2_{q}")
                   for q, ch in enumerate(BN2_CHUNKS)]
            scale1 = consts.tile([128, HT], f32)
            bias1 = consts.tile([128, HT], f32)
            scale2 = consts.tile([128, HT], f32)
            bias2 = consts.tile([128, HT], f32)
            tmps = [consts.tile([128, 12], f32, name=f"tmp{i}")
                    for i in range(3)]
            gvec = consts.tile([1, B_SH], f32)
            gb_sb = consts.tile([128, B_SH], bf16)

            def ag_reduce(ag_t, tot_ap):
                """Sum [128, 8, ...] over ranks into tot (3-level tree)."""
                nc.vector.tensor_add(ag_t[:, 0:4], ag_t[:, 0:4], ag_t[:, 4:8])
                nc.vector.tensor_add(ag_t[:, 0:2], ag_t[:, 0:2], ag_t[:, 2:4])
                nc.vector.tensor_add(tot_ap, ag_t[:, 0], ag_t[:, 1])

            def bn_coeffs(sums, sqs, scl, bia, gbase, bbase, off, ch, t0, t1, t2):
                # mu = sums/B ; var = sqs/B - mu^2
                nc.vector.tensor_scalar_mul(t0[:, 0:ch], sums, 1.0 / B_FULL)
                nc.vector.tensor_scalar_mul(t1[:, 0:ch], sqs, 1.0 / B_FULL)
                nc.vector.tensor_mul(t2[:, 0:ch], t0[:, 0:ch], t0[:, 0:ch])
                nc.vector.tensor_sub(t1[:, 0:ch], t1[:, 0:ch], t2[:, 0:ch])
                nc.scalar.activation(t1[:, 0:ch], t1[:, 0:ch], AFT.Sqrt,
                                     bias=eps_col[:])
                nc.vector.reciprocal(t1[:, 0:ch], t1[:, 0:ch])
                nc.vector.tensor_mul(scl[:, off:off + ch],
                                     gbase[:, off:off + ch], t1[:, 0:ch])
                nc.vector.tensor_mul(t2[:, 0:ch], t0[:, 0:ch],
                                     scl[:, off:off + ch])
                nc.vector.tensor_sub(bia[:, off:off + ch],
                                     bbase[:, off:off + ch], t2[:, 0:ch])

            # ---------------- P1: z = x@W1/S in 3 fp8-DR passes -------------
            zp = tc.tile_pool(name="z", bufs=1)
            z_cm = zp.__enter__()
            z_sb = z_cm.tile([128, HT, B_SH], bf16)

            sqp_cm = tc.tile_pool(name="sq", bufs=3)
            sqp = sqp_cm.__enter__()

            with tc.tile_pool(name="xt", bufs=1) as xtp, \
                 tc.tile_pool(name="w1", bufs=2) as w1p, \
                 tc.tile_pool(name="ps1", bufs=4, space="PSUM") as pp1, \
                 tc.tile_pool(name="psn", bufs=1, space="PSUM") as ppn:
                n2ps = [ppn.tile([1, 512], f32, tag=f"n2_{i}", name=f"n2_{i}")
                        for i in range(2)]

                xts = {}
                for t in "cba":
                    xts[t] = xtp.tile([128, KP, 2, B_SH], fp8, name=f"x_{t}",
                                      tag=f"x_{t}")
                # pass order c, b, a: prefetch next pass' x during current
                nc.sync.dma_start(out=xts["c"][:], in_=x_in[2][:])
                nc.sync.dma_start(out=xts["b"][:], in_=x_in[1][:])

                def load_w1(term, hg):
                    w1t = w1p.tile([128, KP, 2, 256], fp8, tag="w1t")
                    nc.scalar.dma_start(out=w1t[:], in_=W1p[:, hg, term])
                    return w1t

                relu_done = [False] * len(BN1_CHUNKS)

                def bn1_finish_chunk(q):
                    """AllGather chunk q stats, compute coeffs, relu + norm."""
                    ch = BN1_CHUNKS[q]
                    f = int(bn1_first[q])
                    nc.sync.dma_start(out=cc1_ins[q][:], in_=st1[q][:])
                    nc.gpsimd.collective_compute(
                        "AllGather", mybir.AluOpType.bypass,
                        replica_groups=GROUPS,
                        ins=[cc1_ins[q][:]], outs=[cc1_outs[q][:]])
                    nc.sync.dma_start(
                        out=ag1[q][:],
                        in_=cc1_outs[q].rearrange("r p c x -> p r c x"))
                    ag_reduce(ag1[q], st1[q][:])
                    sums = tmps[0]
                    sqs = tmps[1]
                    nc.vector.tensor_add(sums[:, 0:ch], st1[q][:, :, 0],
                                         st1[q][:, :, 1])
                    nc.vector.tensor_add(sqs[:, 0:ch], st1[q][:, :, 2],
                                         st1[q][:, :, 3])
                    bn_coeffs(sums[:, 0:ch], sqs[:, 0:ch], scale1, bias1,
                              g1, bt1, f, ch, tmps[2], sqs, sums)
                    # relu in place + row-norm accumulation for this chunk
                    for i in range(ch):
                        ht = f + i
                        nc.scalar.activation(
                            z_sb[:, ht, :], z_sb[:, ht, :], AFT.Relu,
                            bias=bias1[:, ht:ht + 1],
                            scale=scale1[:, ht:ht + 1])
                        sq = sqp.tile([128, B_SH], bf16, tag="sqn")
                        nc.vector.tensor_mul(sq[:], z_sb[:, ht, :],
                                             z_sb[:, ht, :])
                        for bc in range(2):
                            nc.tensor.matmul(
                                n2ps[bc][:], ones_bf[:],
                                sq[:, bc * 512:(bc + 1) * 512],
                                start=(ht == 0), stop=(ht == HT - 1))
                    relu_done[q] = True

                for pi, t in enumerate("cba"):
                    term = {"a": 0, "b": 1, "c": 2}[t]
                    if t == "b":
                        nc.sync.dma_start(out=xts["a"][:], in_=x_in[0][:])
                    w1_next = load_w1(term, 0)
                    for hg in range(HG):
                        w1t = w1_next
                        if hg + 1 < HG:
                            w1_next = load_w1(term, hg + 1)
                        elif pi < 2:
                            w1_next = load_w1({"c": 1, "b": 0}[t], 0)
                        for hh in range(2):
                            ht = hg * 2 + hh
                            for bc in range(2):
                                ps = pp1.tile([128, 512], f32, tag="ps")
                                for kp in range(KP):
                                    nc.tensor.matmul(
                                        ps[:],
                                        w1t[:, kp, :, hh * 128:(hh + 1) * 128],
                                        xts[t][:, kp, :,
                                               bc * 512:(bc + 1) * 512],
                                        start=(kp == 0), stop=(kp == KP - 1),
                                        perf_mode=DR)
                                if t == "c":
                                    nc.scalar.activation(
                                        z_sb[:, ht, bc * 512:(bc + 1) * 512],
                                        ps[:], AFT.Copy, scale=1.0 / S_W)
                                else:
                                    q, i = bn1_chunk_of(ht)
                                    acc = (dict(accum_out=st1[q][:, i, bc:bc + 1])
                                           if t == "a" else {})
                                    nc.vector.scalar_tensor_tensor(
                                        out=z_sb[:, ht, bc * 512:(bc + 1) * 512],
                                        in0=ps[:], scalar=1.0 / S_W,
                                        in1=z_sb[:, ht, bc * 512:(bc + 1) * 512],
                                        op0=ALU.mult, op1=ALU.add, **acc)
                                    if t == "a":
                                        sq = sqp.tile([128, 512], bf16,
                                                      tag="sq1")
                                        nc.scalar.activation(
                                            sq[:],
                                            z_sb[:, ht, bc * 512:(bc + 1) * 512],
                                            AFT.Square,
                                            accum_out=st1[q][:, i, 2 + bc:3 + bc])
                            if t == "a":
                                q, i = bn1_chunk_of(ht)
                                if i == BN1_CHUNKS[q] - 1:
                                    bn1_finish_chunk(q)

                # ---- g row-scales: g = alpha*min(1, R/|h|) + beta_p
                nc.vector.tensor_copy(gvec[0:1, 0:512], n2ps[0][:])
                nc.vector.tensor_copy(gvec[0:1, 512:1024], n2ps[1][:])

            nc.scalar.activation(gvec[0:1, :], gvec[0:1, :], AFT.Sqrt)
            nc.vector.reciprocal(gvec[0:1, :], gvec[0:1, :])
            nc.vector.tensor_scalar(
                out=gvec[0:1, :], in0=gvec[0:1, :],
                scalar1=R_CLIP, scalar2=1.0, op0=ALU.mult, op1=ALU.min)
            nc.vector.tensor_scalar(
                out=gvec[0:1, :], in0=gvec[0:1, :],
                scalar1=ab_sb[0:1, 0:1], scalar2=ab_sb[0:1, 1:2],
                op0=ALU.mult, op1=ALU.add)
            # broadcast to 128 partitions via rank-1 f32 matmul
            with tc.tile_pool(name="psg", bufs=1, space="PSUM") as ppg:
                gb_ps = ppg.tile([128, B_SH], f32)
                nc.tensor.matmul(gb_ps[:], ones_row[:], gvec[0:1, :],
                                 start=True, stop=True)
                nc.scalar.activation(gb_sb[:], gb_ps[:], AFT.Copy)

            # ---- P2: h2 = g*h (in place), BN2 stats, chunked exchange ----
            def bn2_finish_chunk(q):
                ch = BN2_CHUNKS[q]
                f = int(bn2_first[q])
                nc.sync.dma_start(out=cc2_ins[q][:], in_=st2[q][:])
                nc.gpsimd.collective_compute(
                    "AllGather", mybir.AluOpType.bypass,
                    replica_groups=GROUPS,
                    ins=[cc2_ins[q][:]], outs=[cc2_outs[q][:]])
                nc.sync.dma_start(
                    out=ag2[q][:],
                    in_=cc2_outs[q].rearrange("r p c x -> p r c x"))
                ag_reduce(ag2[q], st2[q][:])
                bn_coeffs(st2[q][:, :, 0], st2[q][:, :, 1], scale2, bias2,
                          g2, bt2, f, ch, tmps[0], tmps[1], tmps[2])
                for i in range(ch):
                    ht = f + i
                    nc.scalar.activation(
                        z_sb[:, ht, :], z_sb[:, ht, :], AFT.Relu,
                        bias=bias2[:, ht:ht + 1], scale=scale2[:, ht:ht + 1])

            for ht in range(HT):
                q, i = bn2_chunk_of(ht)
                nc.vector.scalar_tensor_tensor(
                    out=z_sb[:, ht, :], in0=z_sb[:, ht, :], scalar=1.0,
                    in1=gb_sb[:], op0=ALU.mult, op1=ALU.mult,
                    accum_out=st2[q][:, i, 0:1])
                sq = sqp.tile([128, B_SH], bf16, tag="sq2")
                if ht % 2 == 0:
                    nc.scalar.activation(
                        sq[:], z_sb[:, ht, :], AFT.Square,
                        accum_out=st2[q][:, i, 1:2])
                else:
                    nc.vector.scalar_tensor_tensor(
                        out=sq[:], in0=z_sb[:, ht, :], scalar=1.0,
                        in1=z_sb[:, ht, :], op0=ALU.mult, op1=ALU.mult,
                        accum_out=st2[q][:, i, 1:2])
                if i == BN2_CHUNKS[q] - 1:
                    bn2_finish_chunk(q)

            sqp_cm.__exit__(None, None, None)

            # ---------------- P3: out = h3 @ W2 + b2 ----------------
            with tc.tile_pool(name="w2", bufs=6) as w2p, \
                 tc.tile_pool(name="os", bufs=3) as osp, \
                 tc.tile_pool(name="ps3", bufs=1, space="PSUM") as pp3:
                for half, bts in enumerate((range(0, 4), range(4, 8))):
                    pss = {}
                    for oc, (o0, ow) in enumerate([(0, 512), (512, 488)]):
                        for bt in bts:
                            pss[(oc, bt)] = pp3.tile(
                                [128, 512], f32, tag=f"po{oc}_{bt % 4}",
                                name=f"po{oc}_{bt % 4}")
                    for ht in range(HT):
                        for oc, (o0, ow) in enumerate([(0, 512), (512, 488)]):
                            w2t = w2p.tile([128, 512], bf16, tag="w2t")
                            nc.scalar.dma_start(
                                out=w2t[:, 0:ow],
                                in_=W2p[:, ht, o0:o0 + ow])
                            for bt in bts:
                                nc.tensor.matmul(
                                    pss[(oc, bt)][:, 0:ow],
                                    z_sb[:, ht, bt * 128:(bt + 1) * 128],
                                    w2t[:, 0:ow],
                                    start=(ht == 0), stop=(ht == HT - 1))
                    for oc, (o0, ow) in enumerate([(0, 512), (512, 488)]):
                        for bt in bts:
                            ot = osp.tile([128, 512], f32, tag="ot")
                            nc.vector.tensor_add(
                                ot[:, 0:ow], pss[(oc, bt)][:, 0:ow],
                                b2b[:, o0:o0 + ow])
                            nc.sync.dma_start(
                                out=out[bt * 128:(bt + 1) * 128, o0:o0 + ow],
                                in_=ot[:, 0:ow])

            zp.__exit__(None, None, None)

    nc.compile()
    return nc


def _get_nc(mm_mode=None):
    nc = _BUILD_CACHE.get("nc")
    if nc is None:
        nc = _build()
        _BUILD_CACHE["nc"] = nc
    return nc


MM_MODE = "fp8dr"  # kept for test.py compatibility


def _quantize_terms_x(xT):
    """xT: [D_IN, B] f32 -> (xa, xb, xc) fp8 packed [128, KP, 2, B]."""
    def pack(a):
        return np.ascontiguousarray(
            a.reshape(KP, 2, 128, a.shape[1]).transpose(2, 0, 1, 3))
    xa = xT.astype(F8NP)
    rx = xT - xa.astype(np.float32)
    xb = (rx * S_XB).astype(F8NP)
    xc = (xT / 16.0).astype(F8NP)
    return pack(xa), pack(xb), pack(xc)


def _quantize_terms_w(W1):
    """W1: [D_IN, D_H] f32 -> [128, HG, 3, KP, 2, 256] fp8."""
    WS = W1.astype(np.float32) * np.float32(S_W)
    Wa = WS.astype(F8NP)
    rw = WS - Wa.astype(np.float32)
    Wb = (Wa.astype(np.float32) / S_XB).astype(F8NP)
    Wc = (rw * S_WC).astype(F8NP)
    terms = np.stack([np.asarray(Wa), np.asarray(Wb), np.asarray(Wc)], axis=0)
    # [3, D_IN, D_H] -> [3, KP, 2, 128, HG, 256] -> [128, HG, 3, KP, 2, 256]
    t = terms.reshape(3, KP, 2, 128, HG, 256).transpose(3, 4, 0, 1, 2, 5)
    return np.ascontiguousarray(t)


def kernel(**inputs):
    x = np.asarray(inputs["x"], np.float32)
    g_real = np.asarray(inputs["g_real"], np.float32)
    g_imag = np.asarray(inputs["g_imag"], np.float32)

    # Spectral filter must be (numerically) a delta for the fused fast path.
    ck = _filter_kernel(g_real, g_imag)
    delta = np.zeros_like(ck)
    delta[0] = 1.0
    ck_view = ck.view(np.float32) if ck.dtype == np.complex64 else ck.view(np.float64)
    if not (np.all(np.isfinite(ck_view)) and np.abs(ck - delta).max() < 1e-6):
        a = {k: np.asarray(v) for k, v in inputs.items()}
        return _np_reference(
            a["x"], a["W1"], a["b1"], a["gamma1"], a["beta1"], a["g_real"],
            a["g_imag"], float(a["alpha"][0]), float(a["beta_p"][0]),
            a["gamma2"], a["beta2"], a["W2"], a["b2"])

    from concourse.bass_utils import run_bass_kernel_spmd

    nc = _get_nc()

    def _pt(v):  # [4096] -> [128, 32] partition-major
        return np.ascontiguousarray(
            np.asarray(v, np.float32).reshape(HT, 128).T)

    W2 = np.asarray(inputs["W2"], np.float32)
    # (t p) o -> p t o, bf16
    W2p = np.ascontiguousarray(
        W2.reshape(HT, 128, D_OUT).transpose(1, 0, 2).astype(BF16NP))

    # scale by 8 keeps bf16 rounding behaviour identical; no scale needed
    shared = {
        "W1p": _quantize_terms_w(np.asarray(inputs["W1"], np.float32)),
        "gamma1": _pt(inputs["gamma1"]),
        "beta1": _pt(inputs["beta1"]),
        "gamma2": _pt(inputs["gamma2"]),
        "beta2": _pt(inputs["beta2"]),
        "alpha": np.ascontiguousarray(inputs["alpha"], dtype=np.float32),
        "beta_p": np.ascontiguousarray(inputs["beta_p"], dtype=np.float32),
        "W2p": W2p,
        "b2": np.ascontiguousarray(inputs["b2"], dtype=np.float32),
    }
    in_maps = []
    for c in range(N_CORES):
        sh = dict(shared)
        xT = np.ascontiguousarray(x[c * B_SH:(c + 1) * B_SH, :].T)
        xa, xb, xc = _quantize_terms_x(xT)
        sh["x_a"], sh["x_b"], sh["x_c"] = xa, xb, xc
        in_maps.append(sh)
    res = run_bass_kernel_spmd(nc, in_maps, list(range(N_CORES)))
    return np.concatenate(
        [res.results[c]["out"] for c in range(N_CORES)], axis=0)


# revision 22
# speedup vs baseline: 1.1982x; 1.0905x over previous
"""Trainium2 Bass kernel for the FFT-stacked hyperbolic-BN MLP block.

Math notes (why the device kernel is so simple):

  reference: h  = relu(BN(x@W1 + b1))
             u  = logmap_c(h)          (Poincare ball, c=0.001)
             v  = Re(ifft(fft(u) * H_eff)),  H_eff = exp(L*log(g_real + i g_imag))
             y  = expmap_c(v)
             h3 = relu(BN(alpha*y + beta_p*h))
             out= h3@W2 + b2

  * b1 cancels inside batchnorm (mean subtraction), so it is dropped.
  * With H_eff == 1 (the case whenever g_real==1, g_imag==0, since
    exp(L*log(1)) == 1 exactly in complex fp32), the fft chain is the
    identity:  v == u, and expmap(logmap(h)) collapses to
       y = h * min(1, (1-1e-5)/(sc*|h|)),  so
       alpha*y + beta_p*h = (alpha*min(1,R/|h|) + beta_p) * h =: g(row) * h.
  * More generally the fft chain is a circulant convolution with the real
    kernel Re(ifft(H_eff)); we check at run time that this kernel is a delta
    (it is, for the shipped inputs) and otherwise fall back to a faithful
    numpy implementation of the whole reference.

Device pipeline per core (batch-sharded, 1024 rows/core, 8 cores):

  P1 (three fp8e4m3 DoubleRow passes, residual-corrected):
     z*128 = x(8)Wa + (32(x-x8))(8)(Wa/32) + (x/16)(8)(16(W*128-Wa))
     Terms are host-quantized so every PSUM accumulation carries the same
     2^7 scale; passes run small-to-large (c, b, a) so the bf16 z
     accumulator never rounds a large running value against a small term.
     Pass a's PSUM->SBUF evacuation on DVE carries accum_out (BN1 column
     sums); an ACT Square pass accumulates sum(z^2). DoubleRow processes
     2 k-tiles/instruction at 0.5 cycles/row: 4x the bf16 matmul rate.
  BN1: stats exchanged per column chunk (10|10|8|4 ht tiles) through a
     DRAM AllGather (cheaper than AllReduce in latency and off the
     critical path for all but the last chunk), summed locally on DVE,
     then h = relu(scale*z+bias) in place (bf16).
  Norms: DVE squares + PE ones-matmul partition reduction, interleaved
     into the P1 instruction stream chunk by chunk.
  g row-scales: computed on one partition, broadcast to 128 partitions
     with a rank-1 f32 matmul on PE (no DRAM round trip).
  P2: h2 = g*h on DVE (4x bf16 mode) with fused BN2 column sums;
     sum(h2^2) split across ACT/DVE. BN2 stats exchanged in 3 chunks
     (8|12|12) so P3 can start while later chunks are still in flight.
  P3: out = h3 @ W2 + b2 (bf16, h3 slices stationary), two bt passes of
     8 PSUM banks each; W2 streamed bf16; bias added on DVE during
     evacuation; pass-A rows DMA out while pass B computes.
"""

import os
import sys

sys.path.insert(0, "/opt/trn_rl_repo")

import numpy as np
import ml_dtypes

F8NP = ml_dtypes.float8_e4m3
BF16NP = ml_dtypes.bfloat16

B_FULL = 8192
D_IN = 3072
D_H = 4096
D_OUT = 1000
N_CORES = 8
B_SH = B_FULL // N_CORES          # 1024 rows per core
KT = D_IN // 128                  # 24 k-tiles
KP = KT // 2                      # 12 DoubleRow k-pairs
HT = D_H // 128                   # 32 h-tiles
HG = HT // 2                      # 16 groups of 2 ht (256 cols)
BT = B_SH // 128                  # 8 row-tiles per core

C_CURV = 0.001
EPS = 1e-7
BN_EPS = 1e-5
L_EXP = 100000000
SC = float(np.sqrt(np.float32(C_CURV)))
R_CLIP = float((1.0 - 1e-5) / SC)   # radius above which rows get rescaled

S_W = 128.0                       # power-of-2 scale on W1 (fp8 subnormal guard)
S_XB = 32.0                       # scale on the x residual term
S_WC = 16.0                       # scale on the W residual term

BN1_CHUNKS = [10, 10, 8, 4]       # ht tiles per BN1 stats exchange
BN2_CHUNKS = [6, 12, 14]          # ht tiles per BN2 stats exchange

_BUILD_CACHE = {}


def _filter_kernel(g_real, g_imag):
    """Real circulant kernel of the fft->*H_eff->ifft chain (complex64 math,
    mirroring the reference)."""
    H = g_real.astype(np.complex64) + 1j * g_imag.astype(np.complex64)
    H_eff = np.exp(np.complex64(L_EXP) * np.log(H))
    return np.fft.ifft(H_eff)


def _np_reference(x, W1, b1, gamma1, beta1, g_real, g_imag, alpha, beta_p,
                  gamma2, beta2, W2, b2):
    """Faithful numpy fallback for non-delta spectral filters."""
    def bn(a, gamma, beta):
        mu = a.mean(0)
        var = a.var(0)
        return gamma * (a - mu) / np.sqrt(var + BN_EPS) + beta

    def logmap(h):
        n = np.linalg.norm(h, axis=1, keepdims=True)
        scn = np.clip(SC * n, EPS, 1.0 - 1e-5)
        return np.arctanh(scn) * h / np.maximum(SC * n, EPS)

    def expmap(v):
        n = np.maximum(np.linalg.norm(v, axis=1, keepdims=True), EPS)
        return np.tanh(SC * n) * v / (SC * n)

    h = np.maximum(bn(x @ W1 + b1, gamma1, beta1), 0.0)
    u = logmap(h)
    U = np.fft.fft(u, axis=1)
    H = g_real.astype(np.complex64) + 1j * g_imag.astype(np.complex64)
    H_eff = np.exp(np.complex64(L_EXP) * np.log(H))
    v = np.real(np.fft.ifft(U * H_eff[None, :], axis=1)).astype(np.float32)
    y = expmap(v)
    h2 = alpha * y + beta_p * h
    h3 = np.maximum(bn(h2, gamma2, beta2), 0.0)
    return (h3 @ W2 + b2).astype(np.float32)


def _build():
    import concourse.bacc as bacc
    import concourse.mybir as mybir
    import concourse.tile as tile

    f32 = mybir.dt.float32
    bf16 = mybir.dt.bfloat16
    fp8 = mybir.dt.float8e4
    AFT = mybir.ActivationFunctionType
    ALU = mybir.AluOpType
    DR = mybir.MatmulPerfMode.DoubleRow
    GROUPS = [list(range(N_CORES))]

    nc = bacc.Bacc("TRN2", target_bir_lowering=False, debug=False,
                   num_devices=N_CORES)

    # per-core fp8 term operands, packed [128, kp, j, b]: k = kp*256+j*128+p
    x_in = [nc.dram_tensor(f"x_{t}", [128, KP, 2, B_SH], fp8,
                           kind="ExternalInput") for t in "abc"]
    # shared W1 terms, packed [128, hg, term, kp, j, c(256)]
    W1p = nc.dram_tensor("W1p", [128, HG, 3, KP, 2, 256], fp8,
                         kind="ExternalInput")
    # [4096] -> [128, 32] partition-major
    gamma1 = nc.dram_tensor("gamma1", [128, HT], f32, kind="ExternalInput")
    beta1 = nc.dram_tensor("beta1", [128, HT], f32, kind="ExternalInput")
    gamma2 = nc.dram_tensor("gamma2", [128, HT], f32, kind="ExternalInput")
    beta2 = nc.dram_tensor("beta2", [128, HT], f32, kind="ExternalInput")
    alpha_e = nc.dram_tensor("alpha", [1], f32, kind="ExternalInput")
    beta_p_e = nc.dram_tensor("beta_p", [1], f32, kind="ExternalInput")
    W2p = nc.dram_tensor("W2p", [128, HT, D_OUT], bf16, kind="ExternalInput")
    b2 = nc.dram_tensor("b2", [D_OUT], f32, kind="ExternalInput")
    out = nc.dram_tensor("out", [B_SH, D_OUT], f32, kind="ExternalOutput")

    # BN stats exchange buffers (DRAM AllGather staging)
    cc1_ins = [nc.dram_tensor(f"cc1_in{q}", [128, ch, 4], f32)
               for q, ch in enumerate(BN1_CHUNKS)]
    cc1_outs = [nc.dram_tensor(f"cc1_out{q}", [N_CORES, 128, ch, 4], f32,
                               addr_space="Shared")
                for q, ch in enumerate(BN1_CHUNKS)]
    cc2_ins = [nc.dram_tensor(f"cc2_in{q}", [128, ch, 2], f32)
               for q, ch in enumerate(BN2_CHUNKS)]
    cc2_outs = [nc.dram_tensor(f"cc2_out{q}", [N_CORES, 128, ch, 2], f32,
                               addr_space="Shared")
                for q, ch in enumerate(BN2_CHUNKS)]

    bn1_first = np.cumsum([0] + BN1_CHUNKS)[:-1]
    bn2_first = np.cumsum([0] + BN2_CHUNKS)[:-1]

    def bn1_chunk_of(ht):
        for q, f in enumerate(bn1_first):
            if f <= ht < f + BN1_CHUNKS[q]:
                return q, ht - f
        raise AssertionError

    def bn2_chunk_of(ht):
        for q, f in enumerate(bn2_first):
            if f <= ht < f + BN2_CHUNKS[q]:
                return q, ht - f
        raise AssertionError

    with tile.TileContext(nc) as tc:
        with tc.tile_pool(name="consts", bufs=1) as consts:
            g1 = consts.tile([128, HT], f32)
            bt1 = consts.tile([128, HT], f32)
            g2 = consts.tile([128, HT], f32)
            bt2 = consts.tile([128, HT], f32)
            nc.sync.dma_start(out=g1[:], in_=gamma1[:])
            nc.sync.dma_start(out=bt1[:], in_=beta1[:])
            nc.sync.dma_start(out=g2[:], in_=gamma2[:])
            nc.sync.dma_start(out=bt2[:], in_=beta2[:])
            b2b = consts.tile([128, D_OUT], f32)
            nc.sync.dma_start(out=b2b[:], in_=b2[None, :].to_broadcast([128, D_OUT]))
            ab_sb = consts.tile([1, 2], f32)
            nc.sync.dma_start(out=ab_sb[0:1, 0:1], in_=alpha_e[None, :])
            nc.sync.dma_start(out=ab_sb[0:1, 1:2], in_=beta_p_e[None, :])
            ones_f32 = consts.tile([128, 1], f32)
            nc.vector.memset(ones_f32[:], 1.0)
            ones_bf = consts.tile([128, 1], bf16)
            nc.scalar.activation(ones_bf[:], ones_f32[:], AFT.Identity)
            ones_row = consts.tile([1, 128], f32)
            nc.vector.memset(ones_row[:], 1.0)
            eps_col = consts.tile([128, 1], f32)
            nc.vector.memset(eps_col[:], BN_EPS)

            st1 = [consts.tile([128, ch, 4], f32, name=f"st1_{q}")
                   for q, ch in enumerate(BN1_CHUNKS)]
            st2 = [consts.tile([128, ch, 2], f32, name=f"st2_{q}")
                   for q, ch in enumerate(BN2_CHUNKS)]
            ag1 = [consts.tile([128, N_CORES, ch, 4], f32, name=f"ag1_{q}")
                   for q, ch in enumerate(BN1_CHUNKS)]
            ag2 = [consts.tile([128, N_CORES, ch, 2], f32, name=f"ag2_{q}")
                   for q, ch in enumerate(BN2_CHUNKS)]
            scale1 = consts.tile([128, HT], f32)
            bias1 = consts.tile([128, HT], f32)
            scale2 = consts.tile([128, HT], f32)
            bias2 = consts.tile([128, HT], f32)
            tmps = [consts.tile([128, 16], f32, name=f"tmp{i}")
                    for i in range(3)]
            gvec = consts.tile([1, B_SH], f32)
            gb_sb = consts.tile([128, B_SH], bf16)

            def ag_reduce(ag_t, tot_ap):
                """Sum [128, 8, ...] over ranks into tot (3-level tree)."""
                nc.vector.tensor_add(ag_t[:, 0:4], ag_t[:, 0:4], ag_t[:, 4:8])
                nc.vector.tensor_add(ag_t[:, 0:2], ag_t[:, 0:2], ag_t[:, 2:4])
                nc.vector.tensor_add(tot_ap, ag_t[:, 0], ag_t[:, 1])

            def bn_coeffs(sums, sqs, scl, bia, gbase, bbase, off, ch, t0, t1, t2):
                # mu = sums/B ; var = sqs/B - mu^2
                nc.vector.tensor_scalar_mul(t0[:, 0:ch], sums, 1.0 / B_FULL)
                nc.vector.tensor_scalar_mul(t1[:, 0:ch], sqs, 1.0 / B_FULL)
                nc.vector.tensor_mul(t2[:, 0:ch], t0[:, 0:ch], t0[:, 0:ch])
                nc.vector.tensor_sub(t1[:, 0:ch], t1[:, 0:ch], t2[:, 0:ch])
                nc.scalar.activation(t1[:, 0:ch], t1[:, 0:ch], AFT.Sqrt,
                                     bias=eps_col[:])
                nc.vector.reciprocal(t1[:, 0:ch], t1[:, 0:ch])
                nc.vector.tensor_mul(scl[:, off:off + ch],
                                     gbase[:, off:off + ch], t1[:, 0:ch])
                nc.vector.tensor_mul(t2[:, 0:ch], t0[:, 0:ch],
                                     scl[:, off:off + ch])
                nc.vector.tensor_sub(bia[:, off:off + ch],
                                     bbase[:, off:off + ch], t2[:, 0:ch])

            # ---------------- P1: z = x@W1/S in 3 fp8-DR passes -------------
            zp = tc.tile_pool(name="z", bufs=1)
            z_cm = zp.__enter__()
            z_sb = z_cm.tile([128, HT, B_SH], bf16)

            sqp_cm = tc.tile_pool(name="sq", bufs=3)
            sqp = sqp_cm.__enter__()

            # W2 preload (first output-column half during P1; rest later)
            w2sp_cm = tc.tile_pool(name="w2s", bufs=1)
            w2sp = w2sp_cm.__enter__()
            w2_sb = [w2sp.tile([128, HT, 512], bf16, name="w2h0"), None]

            with tc.tile_pool(name="xt", bufs=2) as xtp, \
                 tc.tile_pool(name="w1", bufs=3) as w1p, \
                 tc.tile_pool(name="ps1", bufs=4, space="PSUM") as pp1, \
                 tc.tile_pool(name="psn", bufs=1, space="PSUM") as ppn:
                n2ps = [ppn.tile([1, 512], f32, tag=f"n2_{i}", name=f"n2_{i}")
                        for i in range(2)]

                xts = {}
                xts["c"] = xtp.tile([128, KP, 2, B_SH], fp8, tag="x", name="xt_c")
                nc.sync.dma_start(out=xts["c"][:], in_=x_in[2][:])

                def load_w1(term, hg):
                    w1t = w1p.tile([128, KP, 2, 256], fp8, tag="w1t")
                    nc.scalar.dma_start(out=w1t[:], in_=W1p[:, hg, term])
                    return w1t

                pending_norm = []  # ht tiles relu'd but norm work deferred

                def emit_norms(upto_len, eng):
                    # squares + partition-reduce matmuls, deferred so the
                    # in-order PE stream trails each chunk's AllGather
                    while len(pending_norm) > upto_len:
                        ht = pending_norm.pop(0)
                        sq = sqp.tile([128, B_SH], bf16, tag="sqn")
                        eng.tensor_mul(sq[:], z_sb[:, ht, :], z_sb[:, ht, :])
                        for bc in range(2):
                            nc.tensor.matmul(
                                n2ps[bc][:], ones_bf[:],
                                sq[:, bc * 512:(bc + 1) * 512],
                                start=(ht == 0), stop=(ht == HT - 1))

                def bn1_finish_chunk(q):
                    """AllGather chunk q stats, compute coeffs, relu+square."""
                    ch = BN1_CHUNKS[q]
                    f = int(bn1_first[q])
                    nc.sync.dma_start(out=cc1_ins[q][:], in_=st1[q][:])
                    nc.gpsimd.collective_compute(
                        "AllGather", mybir.AluOpType.bypass,
                        replica_groups=GROUPS,
                        ins=[cc1_ins[q][:]], outs=[cc1_outs[q][:]])
                    nc.sync.dma_start(
                        out=ag1[q][:],
                        in_=cc1_outs[q].rearrange("r p c x -> p r c x"))
                    ag_reduce(ag1[q], st1[q][:])
                    sums = tmps[0]
                    sqs = tmps[1]
                    nc.vector.tensor_add(sums[:, 0:ch], st1[q][:, :, 0],
                                         st1[q][:, :, 1])
                    nc.vector.tensor_add(sqs[:, 0:ch], st1[q][:, :, 2],
                                         st1[q][:, :, 3])
                    bn_coeffs(sums[:, 0:ch], sqs[:, 0:ch], scale1, bias1,
                              g1, bt1, f, ch, tmps[2], sqs, sums)
                    # relu in place; squares + norm matmuls are deferred
                    for i in range(ch):
                        ht = f + i
                        nc.scalar.activation(
                            z_sb[:, ht, :], z_sb[:, ht, :], AFT.Relu,
                            bias=bias1[:, ht:ht + 1],
                            scale=scale1[:, ht:ht + 1])
                        pending_norm.append(ht)

                for pi, t in enumerate("cba"):
                    term = {"a": 0, "b": 1, "c": 2}[t]
                    w1_next = load_w1(term, 0)
                    if t == "c":
                        xts["b"] = xtp.tile([128, KP, 2, B_SH], fp8, tag="x", name="xt_b")
                        nc.sync.dma_start(out=xts["b"][:], in_=x_in[1][:])
                    if t == "b":
                        xts["a"] = xtp.tile([128, KP, 2, B_SH], fp8, tag="x", name="xt_a")
                        nc.sync.dma_start(out=xts["a"][:], in_=x_in[0][:])
                    for hg in range(HG):
                        w1t = w1_next
                        if hg + 1 < HG:
                            w1_next = load_w1(term, hg + 1)
                        elif pi < 2:
                            w1_next = load_w1({"c": 1, "b": 0}[t], 0)
                        if t == "a" and hg % 4 == 1:
                            # stream first W2 half in during the last pass
                            q4 = (hg - 1) // 4
                            nc.sync.dma_start(
                                out=w2_sb[0][:, q4 * 8:(q4 + 1) * 8],
                                in_=W2p[:, q4 * 8:(q4 + 1) * 8, 0:512])
                        for hh in range(2):
                            ht = hg * 2 + hh
                            if t == "a":
                                emit_norms(7, nc.gpsimd)
                            for bc in range(2):
                                ps = pp1.tile([128, 512], f32, tag="ps")
                                for kp in range(KP):
                                    nc.tensor.matmul(
                                        ps[:],
                                        w1t[:, kp, :, hh * 128:(hh + 1) * 128],
                                        xts[t][:, kp, :,
                                               bc * 512:(bc + 1) * 512],
                                        start=(kp == 0), stop=(kp == KP - 1),
                                        perf_mode=DR)
                                if t == "c":
                                    nc.scalar.activation(
                                        z_sb[:, ht, bc * 512:(bc + 1) * 512],
                                        ps[:], AFT.Copy, scale=1.0 / S_W)
                                else:
                                    q, i = bn1_chunk_of(ht)
                                    acc = (dict(accum_out=st1[q][:, i, bc:bc + 1])
                                           if t == "a" else {})
                                    nc.vector.scalar_tensor_tensor(
                                        out=z_sb[:, ht, bc * 512:(bc + 1) * 512],
                                        in0=ps[:], scalar=1.0 / S_W,
                                        in1=z_sb[:, ht, bc * 512:(bc + 1) * 512],
                                        op0=ALU.mult, op1=ALU.add, **acc)
                                    if t == "a":
                                        sq = sqp.tile([128, 512], bf16,
                                                      tag="sq1")
                                        nc.scalar.activation(
                                            sq[:],
                                            z_sb[:, ht, bc * 512:(bc + 1) * 512],
                                            AFT.Square,
                                            accum_out=st1[q][:, i, 2 + bc:3 + bc])
                            if t == "a":
                                q, i = bn1_chunk_of(ht)
                                if i == BN1_CHUNKS[q] - 1:
                                    bn1_finish_chunk(q)
                emit_norms(0, nc.vector)

                # ---- g row-scales: g = alpha*min(1, R/|h|) + beta_p
                nc.vector.tensor_copy(gvec[0:1, 0:512], n2ps[0][:])
                nc.vector.tensor_copy(gvec[0:1, 512:1024], n2ps[1][:])

            nc.scalar.activation(gvec[0:1, :], gvec[0:1, :], AFT.Sqrt)
            nc.vector.reciprocal(gvec[0:1, :], gvec[0:1, :])
            nc.vector.tensor_scalar(
                out=gvec[0:1, :], in0=gvec[0:1, :],
                scalar1=R_CLIP, scalar2=1.0, op0=ALU.mult, op1=ALU.min)
            nc.vector.tensor_scalar(
                out=gvec[0:1, :], in0=gvec[0:1, :],
                scalar1=ab_sb[0:1, 0:1], scalar2=ab_sb[0:1, 1:2],
                op0=ALU.mult, op1=ALU.add)
            # broadcast to 128 partitions via rank-1 f32 matmul
            with tc.tile_pool(name="psg", bufs=1, space="PSUM") as ppg:
                gb_ps = ppg.tile([128, B_SH], f32)
                nc.tensor.matmul(gb_ps[:], ones_row[:], gvec[0:1, :],
                                 start=True, stop=True)
                nc.scalar.activation(gb_sb[:], gb_ps[:], AFT.Copy)

            # second W2 half loads now that the x tiles' SBUF is free
            w2sp2_cm = tc.tile_pool(name="w2s2", bufs=1)
            w2sp2 = w2sp2_cm.__enter__()
            w2_sb[1] = w2sp2.tile([128, HT, 488], bf16, name="w2h1", tag="w2h1")
            for q4 in range(4):
                nc.sync.dma_start(
                    out=w2_sb[1][:, q4 * 8:(q4 + 1) * 8],
                    in_=W2p[:, q4 * 8:(q4 + 1) * 8, 512:1000])

            # ---- P2: h2 = g*h (in place), BN2 stats, chunked exchange ----
            def bn2_finish_chunk(q):
                ch = BN2_CHUNKS[q]
                f = int(bn2_first[q])
                nc.sync.dma_start(out=cc2_ins[q][:], in_=st2[q][:])
                nc.gpsimd.collective_compute(
                    "AllGather", mybir.AluOpType.bypass,
                    replica_groups=GROUPS,
                    ins=[cc2_ins[q][:]], outs=[cc2_outs[q][:]])
                nc.sync.dma_start(
                    out=ag2[q][:],
                    in_=cc2_outs[q].rearrange("r p c x -> p r c x"))
                ag_reduce(ag2[q], st2[q][:])
                bn_coeffs(st2[q][:, :, 0], st2[q][:, :, 1], scale2, bias2,
                          g2, bt2, f, ch, tmps[0], tmps[1], tmps[2])
                for i in range(ch):
                    ht = f + i
                    nc.scalar.activation(
                        z_sb[:, ht, :], z_sb[:, ht, :], AFT.Relu,
                        bias=bias2[:, ht:ht + 1], scale=scale2[:, ht:ht + 1])

            for ht in range(HT):
                q, i = bn2_chunk_of(ht)
                nc.vector.scalar_tensor_tensor(
                    out=z_sb[:, ht, :], in0=z_sb[:, ht, :], scalar=1.0,
                    in1=gb_sb[:], op0=ALU.mult, op1=ALU.mult,
                    accum_out=st2[q][:, i, 0:1])
                sq = sqp.tile([128, B_SH], bf16, tag="sq2")
                nc.scalar.activation(
                    sq[:], z_sb[:, ht, :], AFT.Square,
                    accum_out=st2[q][:, i, 1:2])
                if i == BN2_CHUNKS[q] - 1:
                    bn2_finish_chunk(q)

            # ---------------- P3: out = h3 @ W2 + b2 ----------------
            with tc.tile_pool(name="os", bufs=3) as osp, \
                 tc.tile_pool(name="ps3", bufs=1, space="PSUM") as pp3:
                for half, bts in enumerate((range(0, 4), range(4, 8))):
                    pss = {}
                    for oc, (o0, ow) in enumerate([(0, 512), (512, 488)]):
                        for bt in bts:
                            pss[(oc, bt)] = pp3.tile(
                                [128, 512], f32, tag=f"po{oc}_{bt % 4}",
                                name=f"po{oc}_{bt % 4}")
                    for ht in range(HT):
                        for oc, (o0, ow) in enumerate([(0, 512), (512, 488)]):
                            for bt in bts:
                                nc.tensor.matmul(
                                    pss[(oc, bt)][:, 0:ow],
                                    z_sb[:, ht, bt * 128:(bt + 1) * 128],
                                    w2_sb[oc][:, ht, :],
                                    start=(ht == 0), stop=(ht == HT - 1))
                    for oc, (o0, ow) in enumerate([(0, 512), (512, 488)]):
                        for bt in bts:
                            ot = osp.tile([128, 512], f32, tag="ot")
                            nc.vector.tensor_add(
                                ot[:, 0:ow], pss[(oc, bt)][:, 0:ow],
                                b2b[:, o0:o0 + ow])
                            nc.sync.dma_start(
                                out=out[bt * 128:(bt + 1) * 128, o0:o0 + ow],
                                in_=ot[:, 0:ow])

            w2sp2_cm.__exit__(None, None, None)
            w2sp_cm.__exit__(None, None, None)
            sqp_cm.__exit__(None, None, None)
            zp.__exit__(None, None, None)

    nc.compile()
    return nc


def _get_nc(mm_mode=None):
    nc = _BUILD_CACHE.get("nc")
    if nc is None:
        nc = _build()
        _BUILD_CACHE["nc"] = nc
    return nc


MM_MODE = "fp8dr"  # kept for test.py compatibility


def _quantize_terms_x(xT):
    """xT: [D_IN, B] f32 -> (xa, xb, xc) fp8 packed [128, KP, 2, B]."""
    def pack(a):
        return np.ascontiguousarray(
            a.reshape(KP, 2, 128, a.shape[1]).transpose(2, 0, 1, 3))
    xa = xT.astype(F8NP)
    rx = xT - xa.astype(np.float32)
    xb = (rx * S_XB).astype(F8NP)
    xc = (xT / 16.0).astype(F8NP)
    return pack(xa), pack(xb), pack(xc)


def _quantize_terms_w(W1):
    """W1: [D_IN, D_H] f32 -> [128, HG, 3, KP, 2, 256] fp8."""
    WS = W1.astype(np.float32) * np.float32(S_W)
    Wa = WS.astype(F8NP)
    rw = WS - Wa.astype(np.float32)
    Wb = (Wa.astype(np.float32) / S_XB).astype(F8NP)
    Wc = (rw * S_WC).astype(F8NP)
    terms = np.stack([np.asarray(Wa), np.asarray(Wb), np.asarray(Wc)], axis=0)
    # [3, D_IN, D_H] -> [3, KP, 2, 128, HG, 256] -> [128, HG, 3, KP, 2, 256]
    t = terms.reshape(3, KP, 2, 128, HG, 256).transpose(3, 4, 0, 1, 2, 5)
    return np.ascontiguousarray(t)


def kernel(**inputs):
    x = np.asarray(inputs["x"], np.float32)
    g_real = np.asarray(inputs["g_real"], np.float32)
    g_imag = np.asarray(inputs["g_imag"], np.float32)

    # Spectral filter must be (numerically) a delta for the fused fast path.
    ck = _filter_kernel(g_real, g_imag)
    delta = np.zeros_like(ck)
    delta[0] = 1.0
    ck_view = ck.view(np.float32) if ck.dtype == np.complex64 else ck.view(np.float64)
    if not (np.all(np.isfinite(ck_view)) and np.abs(ck - delta).max() < 1e-6):
        a = {k: np.asarray(v) for k, v in inputs.items()}
        return _np_reference(
            a["x"], a["W1"], a["b1"], a["gamma1"], a["beta1"], a["g_real"],
            a["g_imag"], float(a["alpha"][0]), float(a["beta_p"][0]),
            a["gamma2"], a["beta2"], a["W2"], a["b2"])

    from concourse.bass_utils import run_bass_kernel_spmd

    nc = _get_nc()

    def _pt(v):  # [4096] -> [128, 32] partition-major
        return np.ascontiguousarray(
            np.asarray(v, np.float32).reshape(HT, 128).T)

    W2 = np.asarray(inputs["W2"], np.float32)
    # (t p) o -> p t o, bf16
    W2p = np.ascontiguousarray(
        W2.reshape(HT, 128, D_OUT).transpose(1, 0, 2).astype(BF16NP))

    # scale by 8 keeps bf16 rounding behaviour identical; no scale needed
    shared = {
        "W1p": _quantize_terms_w(np.asarray(inputs["W1"], np.float32)),
        "gamma1": _pt(inputs["gamma1"]),
        "beta1": _pt(inputs["beta1"]),
        "gamma2": _pt(inputs["gamma2"]),
        "beta2": _pt(inputs["beta2"]),
        "alpha": np.ascontiguousarray(inputs["alpha"], dtype=np.float32),
        "beta_p": np.ascontiguousarray(inputs["beta_p"], dtype=np.float32),
        "W2p": W2p,
        "b2": np.ascontiguousarray(inputs["b2"], dtype=np.float32),
    }
    in_maps = []
    for c in range(N_CORES):
        sh = dict(shared)
        xT = np.ascontiguousarray(x[c * B_SH:(c + 1) * B_SH, :].T)
        xa, xb, xc = _quantize_terms_x(xT)
        sh["x_a"], sh["x_b"], sh["x_c"] = xa, xb, xc
        in_maps.append(sh)
    res = run_bass_kernel_spmd(nc, in_maps, list(range(N_CORES)))
    return np.concatenate(
        [res.results[c]["out"] for c in range(N_CORES)], axis=0)
